# revision 19
# baseline (speedup 1.0000x reference)
"""ContentGuidedAttention Trainium2 kernel.

Full NxN single-head cross-attention + out-proj + residual + LayerNorm,
for B=4, C=256, H=W=64 (N=4096 tokens), distributed over 8 NeuronCores:
core i handles batch i//2, query-half i%2 (2048 queries, all 4096 keys).
No collectives: K/V are computed redundantly on the two cores sharing a
batch (~5% extra FLOPs).

Layout strategy (all channel-major, zero transposes):
  - the out-projection is folded into V host-side: W_vo = (o_w @ v_w),
    so PV directly yields the projected output; V is scaled by 64 (and
    Q/K weights by 16) to keep fp8e4 out of subnormals, with the scales
    folded back via the exp scale and the denominator ones-vector
  - Q^T, K^T computed as [C, n] (channels on partitions) in fp8e4
  - V' computed token-major [n, C] in fp8e4
  - all projections and attention matmuls run fp8 DoubleRow (K=256
    contraction per MM, ~1.45x bf16 PE rate)
  - S^T = K Q^T as [k, q] psum tiles; exp on ACT -> P^T fp8e4
  - softmax denominator: quarters 0-1 via DVE bf16 chunk-tree,
    quarters 2-3 via fp8 DoubleRow ones-matmuls, all accumulated in
    one [1, q] psum group (the ones carry the 64x V scale)
  - reciprocals and rsqrt run on ACT as exp(-ln x) / exp(-0.5 ln x):
    Ln and Exp share one activation-table set, so no table switches
  - row -> all-partition replication via GpSimd partition broadcast
  - LN per-query-block, overlapped with the next block's attention
"""

import numpy as np

import concourse.bass as bass
import concourse.mybir as mybir
import concourse.tile as tile
from concourse import bacc
from concourse.bass import ds, ts
from concourse.bass_utils import run_bass_kernel_spmd

F32 = mybir.dt.float32
F32R = mybir.dt.float32r
BF16 = mybir.dt.bfloat16
F8 = mybir.dt.float8e4
AF = mybir.ActivationFunctionType
OP = mybir.AluOpType
PM = mybir.MatmulPerfMode

B = 4
C = 256
N = 4096          # tokens per batch
NQ = 2048         # queries per core
QB = 512          # query block
NQB = NQ // QB    # 4
NKC = N // 128    # 32 key chunks
NKR = 4           # key ranges (1024 keys each) for K^T / V tiles
SQK = 16.0        # host-side scale on wq/wk (fp8 subnormal avoidance)
SV = 64.0         # host-side scale on wvo
SCALE = (C // 8) ** -0.5 / (SQK * SQK)
LN_EPS = 1e-5

_CACHE = {}


def _build_nc(dbg=False):
    nc = bacc.Bacc("TRN2", target_bir_lowering=False, debug=False)

    low_d = nc.declare_dram_parameter("low", [C, NQ], F32R, isOutput=False)
    lowq_d = nc.declare_dram_parameter("lowq", [C, NQ], F8, isOutput=False)
    high_d = nc.declare_dram_parameter("high", [C, N], F8, isOutput=False)
    # weights pre-transposed [c_in, c_out]; wvo = (o_w @ v_w).T * 64
    wq_d = nc.declare_dram_parameter("wq", [C, C], F8, isOutput=False)
    wk_d = nc.declare_dram_parameter("wk", [C, C], F8, isOutput=False)
    wvo_d = nc.declare_dram_parameter("wvo", [C, C], F8, isOutput=False)
    # qb16, kb16, ob_eff, lng, lnb prepacked host-side as [128, 10]
    pvec_d = nc.declare_dram_parameter("pvec", [128, 10], F32, isOutput=False)
    out_d = nc.declare_dram_parameter("out", [C, NQ], F32, isOutput=True)
    dbg_d = {}
    if dbg:
        for nm, shp, dt_ in [
            ("dbg_rcp", [1, 512], F32), ("dbg_mu", [1, 512], F32),
            ("dbg_var", [1, 512], F32), ("dbg_rstd", [1, 512], F32),
            ("dbg_pps", [128, 2, QB], F32),
            ("dbg_qt", [128, 2, QB], F8), ("dbg_kt", [128, 2, 1024], F8),
            ("dbg_v", [128, 8, C], F8), ("dbg_pt", [128, 8, QB], F8),
        ]:
            dbg_d[nm] = nc.declare_dram_parameter(nm, shp, dt_, isOutput=True)

    with tile.TileContext(nc) as tc:
        with (
            tc.tile_pool(name="persist", bufs=1) as pp,
            tc.tile_pool(name="high", bufs=3) as high_pool,
            tc.tile_pool(name="pt", bufs=9) as pt_pool,
            tc.tile_pool(name="yt", bufs=2) as yt_pool,
            tc.tile_pool(name="scratch", bufs=2) as scr_pool,
            tc.tile_pool(name="rowscr", bufs=1) as row_pool,
            tc.tile_pool(name="outsb", bufs=2) as out_pool,
            tc.tile_pool(name="st_ps", bufs=2, space="PSUM") as st_ps,
            tc.tile_pool(name="acc_ps", bufs=3, space="PSUM") as acc_ps,
            tc.tile_pool(name="row_ps", bufs=1, space="PSUM") as row_ps,
        ):
            # ---------------- constants / parameters ----------------
            # single-descriptor DMAs: the [256, n] DRAM halves fold into
            # [128, 2, n] SBUF tiles via AP rearrange, one post each
            pvec = pp.tile([128, 10], F32)
            nc.sync.dma_start(out=pvec[:, :], in_=pvec_d[:, :])
            # DMA order matters: hi chunks first (gate the K proj and
            # everything after), then lowq; the big f32 low residual
            # rides the scalar queue after the weights (needed ~25us in)
            hi_sb = [
                pp.tile([128, 2, 1024], F8, name=f"hi{r}", tag=f"hi{r}")
                for r in range(NKR)
            ]
            wk_sb = pp.tile([128, 2, C], F8)
            wq_sb = pp.tile([128, 2, C], F8)
            wvo_sb = pp.tile([128, 2, C], F8)
            for t, d in [(wk_sb, wk_d), (wq_sb, wq_d), (wvo_sb, wvo_d)]:
                nc.scalar.dma_start(
                    out=t[:, :, :],
                    in_=d[:, :].rearrange("(j p) k -> p j k", j=2),
                )
            for r in range(NKR):
                nc.sync.dma_start(
                    out=hi_sb[r][:, :, :],
                    in_=high_d[:, ds(r * 1024, 1024)].rearrange(
                        "(j p) k -> p j k", j=2
                    ),
                )
            lowq_sb = pp.tile([128, 2, NQ], F8)
            nc.sync.dma_start(
                out=lowq_sb[:, :, :],
                in_=lowq_d[:, :].rearrange("(j p) k -> p j k", j=2),
            )
            low_sb = pp.tile([128, 2, NQ], F32R)
            nc.scalar.dma_start(
                out=low_sb[:, :, :],
                in_=low_d[:, :].rearrange("(j p) k -> p j k", j=2),
            )

            # memset cannot emit float32r/fp8; stage in f32 and copy
            stage = pp.tile([128, 128], F32)
            ones128 = pp.tile([128, 1], F32R)    # partition-reduce lhsT (f32r)
            nc.vector.memset(stage[:, 0:1], 1.0)
            nc.vector.tensor_copy(ones128[:, :], stage[:, 0:1])
            # denominator lhsT carries the 64x V scale
            ones_f8 = pp.tile([128, 2, 16], F8)  # fp8 DoubleRow ones (col 0)
            nc.vector.memset(stage[:, 0:32], SV)
            nc.vector.tensor_copy(
                ones_f8[:, :, :], stage[:, 0:32].rearrange("p (a b) -> p a b", a=2)
            )
            epsb = pp.tile([1, 1], F32)          # LN epsilon bias
            nc.vector.memset(epsb[:, :], LN_EPS)

            # HAM warm-up: dummy f32 matmuls fill the DMA-wait window so
            # the PE clock-gate opens before the first real matmul
            warm_ps = row_ps.tile([1, 128], F32, tag="row")
            for w in range(10):
                nc.tensor.matmul(
                    out=warm_ps[:, :], lhsT=stage[:, 0:1], rhs=stage[:, :],
                    start=True, stop=True,
                )

            QBIAS, KBIAS, OBIAS, LNG, LNB = 0, 2, 4, 6, 8

            # ---------------- projections ----------------
            # psum tiles alternate between the st/acc pools so the
            # projections pace at PE speed, not DVE bias-add speed.
            kt_sb = [
                pp.tile([128, 2, 1024], F8, name=f"kt{r}", tag=f"kt{r}")
                for r in range(NKR)
            ]
            v_sb = [
                pp.tile([128, 8, C], F8, name=f"v{r}", tag=f"v{r}")
                for r in range(NKR)
            ]
            qt_all = pp.tile([128, 2, NQ], F8)

            def proj_psum(i, shape):
                pool = (st_ps, acc_ps, acc_ps)[i % 3]
                return pool.tile(
                    shape, F32, tag="st" if i % 3 == 0 else "acc",
                    name=f"pps{i}",
                )

            def k_proj():
                # K^T: out [cout, k] = sum_cin wk[cin, cout] high[cin, k]
                i = 0
                for r in range(NKR):
                    for h in range(2):
                        for c in range(2):
                            kps = proj_psum(i, [128, 512])
                            i += 1
                            nc.tensor.matmul(
                                out=kps[:, :],
                                lhsT=wk_sb[:, :, ds(c * 128, 128)],
                                rhs=hi_sb[r][:, :, ds(h * 512, 512)],
                                start=True, stop=True,
                                perf_mode=PM.DoubleRow,
                            )
                            nc.vector.tensor_scalar_add(
                                out=kt_sb[r][:, c, ds(h * 512, 512)],
                                in0=kps[:, :],
                                scalar1=pvec[:, ds(KBIAS + c, 1)],
                            )

            def q_proj():
                i = 0
                for qb4 in range(NQB):
                    for c in range(2):
                        qps = proj_psum(i, [128, QB])
                        i += 1
                        nc.tensor.matmul(
                            out=qps[:, :],
                            lhsT=wq_sb[:, :, ds(c * 128, 128)],
                            rhs=lowq_sb[:, :, ds(qb4 * QB, QB)],
                            start=True, stop=True,
                            perf_mode=PM.DoubleRow,
                        )
                        nc.vector.tensor_scalar_add(
                            out=qt_all[:, c, ds(qb4 * QB, QB)], in0=qps[:, :],
                            scalar1=pvec[:, ds(QBIAS + c, 1)],
                        )

            def v_proj():
                # V': out [k, cout] = sum_cin high[cin, k] wvo[cin, cout]
                # copies on DVE: ACT is already saturated by block 0 exp
                i = 0
                for r in range(NKR):
                    for u in range(8):
                        vps = proj_psum(i, [128, C])
                        i += 1
                        nc.tensor.matmul(
                            out=vps[:, :],
                            lhsT=hi_sb[r][:, :, ds(u * 128, 128)],
                            rhs=wvo_sb[:, :, :],
                            start=True, stop=True,
                            perf_mode=PM.DoubleRow,
                        )
                        nc.vector.tensor_copy(v_sb[r][:, u, :], vps[:, :])

            # ---------------- main loop over query blocks ----------------

            def attention(b):
                qsl = ds(b * QB, QB)
                quarters = [
                    pt_pool.tile([128, 8, QB], F8, tag="ptq", name=f"ptq{g}")
                    for g in range(4)
                ]
                for si in range(16):
                    sps = st_ps.tile([128, 2, QB], F32, tag="st")
                    for u in range(2):
                        kc = si * 2 + u
                        # DoubleRow: full C=256 contraction in one fp8 MM
                        nc.tensor.matmul(
                            out=sps[:, u, :],
                            lhsT=kt_sb[kc // 8][:, :, ds((kc % 8) * 128, 128)],
                            rhs=qt_all[:, :, qsl],
                            start=True, stop=True,
                            perf_mode=PM.DoubleRow,
                        )
                    nc.scalar.activation(
                        out=quarters[si // 4][:, ds((si % 4) * 2, 2), :],
                        in_=sps[:, :, :],
                        func=AF.Exp,
                        scale=SCALE,
                    )
                return quarters

            def pv(b, quarters):
                pps = []
                for c in range(2):
                    ops = acc_ps.tile([128, QB], F32, tag="acc")
                    for t in range(NKC // 2):
                        # DoubleRow: two adjacent 128-key chunks per fp8 MM
                        nc.tensor.matmul(
                            out=ops[:, :],
                            lhsT=v_sb[t // 4][:, ds((t % 4) * 2, 2), ds(c * 128, 128)],
                            rhs=quarters[t // 4][:, ds((t % 4) * 2, 2), :],
                            start=(t == 0), stop=(t == NKC // 2 - 1),
                            perf_mode=PM.DoubleRow,
                        )
                    pps.append(ops)
                return pps

            def denom(b, quarters):
                # softmax denominator: fp8 DoubleRow ones-matmuls over
                # every quarter pair, one [1, QB] psum accumulation
                # group; the 64x lhsT values fold in the V' scale.
                # Purely exp-gated (no DVE dependency), so the next
                # block's S matmuls aren't stalled behind DVE folds.
                dn_ps = row_ps.tile([1, QB], F32, tag="row")
                for i in range(16):
                    nc.tensor.matmul(
                        out=dn_ps[:, :],
                        lhsT=ones_f8[:, :, 0:1],
                        rhs=quarters[i // 4][:, ds((i % 4) * 2, 2), :],
                        start=(i == 0), stop=(i == 15),
                        perf_mode=PM.DoubleRow,
                    )
                # 1/denom = exp(-ln(denom)) on ACT (same table set as Exp)
                lnrow = row_pool.tile([1, QB], F32, tag="lnrow")
                nc.scalar.activation(
                    out=lnrow[:, :], in_=dn_ps[:, :], func=AF.Ln
                )
                rcprow = row_pool.tile([1, QB], F32, tag="rcprow",
                                       name=f"rcprow{b}")
                nc.scalar.activation(
                    out=rcprow[:, :], in_=lnrow[:, :], func=AF.Exp, scale=-1.0
                )
                rcp_rep = scr_pool.tile([128, QB], F32, tag="rcprep",
                                        name=f"rcprep{b}")
                nc.gpsimd.partition_broadcast(rcp_rep[:, :], rcprow[:, :])
                return rcprow, rcp_rep

            def make_y(b, pps, rcp_rep):
                qsl = ds(b * QB, QB)
                y_sb = yt_pool.tile([128, 2, QB], F32R, tag="y", name=f"y{b}")
                for c in range(2):
                    ysc = scr_pool.tile([128, QB], F32, tag="scr")
                    nc.vector.tensor_mul(
                        out=ysc[:, :], in0=pps[c][:, :], in1=rcp_rep[:, :]
                    )
                    nc.vector.scalar_tensor_tensor(
                        out=y_sb[:, c, :],
                        in0=ysc[:, :],
                        scalar=pvec[:, ds(OBIAS + c, 1)],
                        in1=low_sb[:, c, qsl].bitcast(F32),
                        op0=OP.add, op1=OP.add,
                    )
                return y_sb

            def stats_ln(b, y_sb, rcprow):
                sy_ps = row_ps.tile([1, QB], F32, tag="row")
                for c in range(2):
                    nc.tensor.matmul(
                        out=sy_ps[:, :],
                        lhsT=ones128[:, :],
                        rhs=y_sb[:, c, :],
                        start=(c == 0), stop=(c == 1),
                    )
                murow = row_pool.tile([1, QB], F32, tag="murow")
                nc.vector.tensor_scalar_mul(
                    out=murow[:, :], in0=sy_ps[:, :], scalar1=1.0 / C
                )
                sy2_ps = row_ps.tile([1, QB], F32, tag="row")
                for c in range(2):
                    ysq = scr_pool.tile([128, QB], F32R, tag="ysq")
                    nc.vector.tensor_mul(
                        out=ysq[:, :],
                        in0=y_sb[:, c, :].bitcast(F32),
                        in1=y_sb[:, c, :].bitcast(F32),
                    )
                    nc.tensor.matmul(
                        out=sy2_ps[:, :],
                        lhsT=ones128[:, :],
                        rhs=ysq[:, :],
                        start=(c == 0), stop=(c == 1),
                    )
                # var = E[y^2] - mu^2 ; rstd = exp(-0.5 ln(var + eps))
                varrow = row_pool.tile([1, QB], F32, tag="varrow")
                nc.vector.tensor_scalar_mul(
                    out=varrow[:, :], in0=sy2_ps[:, :], scalar1=1.0 / C
                )
                mu2row = row_pool.tile([1, QB], F32, tag="mu2row")
                nc.vector.tensor_mul(
                    out=mu2row[:, :], in0=murow[:, :], in1=murow[:, :],
                )
                nc.vector.tensor_sub(
                    out=varrow[:, :], in0=varrow[:, :], in1=mu2row[:, :]
                )
                lnv = row_pool.tile([1, QB], F32, tag="lnv")
                nc.scalar.activation(
                    out=lnv[:, :], in_=varrow[:, :], func=AF.Ln, bias=epsb[:, :]
                )
                rstdrow = row_pool.tile([1, QB], F32, tag="rstdrow")
                nc.scalar.activation(
                    out=rstdrow[:, :], in_=lnv[:, :], func=AF.Exp, scale=-0.5
                )
                if dbg_d and b == NQB - 1:
                    nc.sync.dma_start(out=dbg_d["dbg_rcp"][:, :], in_=rcprow[:, :])
                    nc.sync.dma_start(out=dbg_d["dbg_mu"][:, :], in_=murow[:, :])
                    nc.sync.dma_start(out=dbg_d["dbg_var"][:, :],
                                      in_=varrow[:, :])
                    nc.sync.dma_start(out=dbg_d["dbg_rstd"][:, :],
                                      in_=rstdrow[:, :])
                mu_rep = scr_pool.tile([128, QB], F32, tag="murep")
                nc.gpsimd.partition_broadcast(mu_rep[:, :], murow[:, :])
                rs_rep = scr_pool.tile([128, QB], F32, tag="rsrep")
                nc.gpsimd.partition_broadcast(rs_rep[:, :], rstdrow[:, :])
                qsl = ds(b * QB, QB)
                osb = out_pool.tile([128, 2, QB], F32)
                for c in range(2):
                    yn = scr_pool.tile([128, QB], F32, tag="scr")
                    nc.vector.tensor_sub(
                        out=yn[:, :],
                        in0=y_sb[:, c, :].bitcast(F32),
                        in1=mu_rep[:, :],
                    )
                    nc.vector.tensor_mul(
                        out=yn[:, :], in0=yn[:, :], in1=rs_rep[:, :]
                    )
                    nc.vector.tensor_scalar(
                        out=osb[:, c, :], in0=yn[:, :],
                        scalar1=pvec[:, ds(LNG + c, 1)],
                        scalar2=pvec[:, ds(LNB + c, 1)],
                        op0=OP.mult, op1=OP.add,
                    )
                nc.sync.dma_start(
                    out=out_d[:, qsl].rearrange("(j p) q -> p j q", j=2),
                    in_=osb[:, :, :],
                )

            k_proj()
            q_proj()
            v_proj()
            for b in range(NQB):
                quarters = attention(b)
                pps = pv(b, quarters)
                rcprow, rcp_rep = denom(b, quarters)
                y_b = make_y(b, pps, rcp_rep)
                stats_ln(b, y_b, rcprow)
                if dbg_d and b == NQB - 1:
                    nc.sync.dma_start(
                        out=dbg_d["dbg_pps"][:, 0, :], in_=pps[0][:, :]
                    )
                    nc.sync.dma_start(out=dbg_d["dbg_qt"][:, :, :],
                                      in_=qt_all[:, :, 3 * QB:4 * QB])
                    nc.sync.dma_start(
                        out=dbg_d["dbg_kt"][:, :, :], in_=kt_sb[0][:, :, :]
                    )
                    nc.sync.dma_start(
                        out=dbg_d["dbg_v"][:, :, :], in_=v_sb[0][:, :, :]
                    )
                    nc.sync.dma_start(
                        out=dbg_d["dbg_pt"][:, :, :], in_=quarters[3][:, :, :]
                    )

    # Force Exp and Ln to resolve to the one table set containing both
    # (the default chooser alternates exp_and_others <-> natural_log_exp,
    # paying a ~1.3us table load per switch, ~17 loads per kernel).
    import bass_rust as _br
    from concourse.hw_specs import get_activation_tables as _gat

    def _patched_act_loads():
        has_act = any(
            isinstance(i, mybir.InstActivation)
            for blk in nc.main_func.blocks for i in blk.instructions
        )
        if not has_act:
            return
        tables = []
        for name, fns in _gat(nc.m.arch).items():
            if name != "natural_log_exp_and_others":
                fns = fns - {AF.Exp, AF.Ln}
            tables.append((name, fns))
        _br.insert_act_table_loads(nc, tables)

    nc.insert_act_table_loads = _patched_act_loads
    nc.compile()
    return nc


def get_nc(dbg=False):
    key = "nc_dbg" if dbg else "nc"
    if key not in _CACHE:
        _CACHE[key] = _build_nc(dbg)
    return _CACHE[key]


def make_in_maps(low, high, q_w, q_b, k_w, k_b, v_w, v_b, o_w, o_b, ln_g, ln_b):
    import ml_dtypes
    f32 = lambda x: np.ascontiguousarray(np.asarray(x, np.float32))
    f8 = lambda x: np.ascontiguousarray(
        np.asarray(x, np.float32).astype(ml_dtypes.float8_e4m3)
    )
    low_r = np.asarray(low, np.float32).reshape(B, C, N)
    high_r = np.asarray(high, np.float32).reshape(B, C, N)
    # v-bias is exactly equivalent to an out-proj bias shift because the
    # softmax rows sum to one: attn @ (V + 1 vb^T) @ o_w^T = attn @ V @ o_w^T
    # + (o_w @ v_b)^T, so fold it on the host. The out-projection itself
    # folds into V: attn @ V @ o_w^T = attn @ (high_t @ (o_w @ v_w).T).
    o_w = np.asarray(o_w, np.float32)
    v_w = np.asarray(v_w, np.float32)
    ob_eff = np.asarray(o_b, np.float32) + o_w @ np.asarray(v_b, np.float32)
    w_vo = (o_w @ v_w) * SV
    pv_cols = []
    for v in [np.asarray(q_b, np.float32) * SQK,
              np.asarray(k_b, np.float32) * SQK, ob_eff, ln_g, ln_b]:
        pv_cols.append(np.asarray(v, np.float32).reshape(2, 128).T)
    shared = {
        "wq": f8(np.asarray(q_w, np.float32).T * SQK),
        "wk": f8(np.asarray(k_w, np.float32).T * SQK),
        "wvo": f8(w_vo.T),
        "pvec": f32(np.concatenate(pv_cols, axis=1)),
    }
    in_maps = []
    for i in range(8):
        bidx, h = i // 2, i % 2
        in_maps.append({
            "low": f32(low_r[bidx][:, h * NQ:(h + 1) * NQ]),
            "lowq": f8(low_r[bidx][:, h * NQ:(h + 1) * NQ]),
            "high": f8(high_r[bidx]),
            **shared,
        })
    return in_maps


def assemble(results):
    out = np.empty((B, C, N), np.float32)
    for i in range(8):
        bidx, h = i // 2, i % 2
        out[bidx][:, h * NQ:(h + 1) * NQ] = results[i]["out"]
    return out.reshape(B, C, 64, 64)


def kernel(**inputs) -> np.ndarray:
    nc = get_nc()
    in_maps = make_in_maps(**inputs)
    res = run_bass_kernel_spmd(nc, in_maps, core_ids=list(range(8)))
    return assemble(res.results)


if __name__ == "__main__":
    pass


# revision 23
# speedup vs baseline: 1.0639x; 1.0639x over previous
"""ContentGuidedAttention Trainium2 kernel.

Full NxN single-head cross-attention + out-proj + residual + LayerNorm,
for B=4, C=256, H=W=64 (N=4096 tokens), distributed over 8 NeuronCores:
core i handles batch i//2, query-half i%2 (2048 queries, all 4096 keys).
No collectives: K/V are computed redundantly on the two cores sharing a
batch (~5% extra FLOPs).

Layout strategy (all channel-major, zero transposes):
  - the out-projection is folded into V host-side: W_vo = (o_w @ v_w),
    so PV directly yields the projected output; V is scaled by 64 (and
    Q/K weights by 16) to keep fp8e4 out of subnormals, with the scales
    folded back via the exp scale and the denominator ones-vector
  - Q^T, K^T computed as [C, n] (channels on partitions) in fp8e4
  - V' computed token-major [n, C] in fp8e4
  - all projections and attention matmuls run fp8 DoubleRow (K=256
    contraction per MM, ~1.45x bf16 PE rate)
  - S^T = K Q^T as [k, q] psum tiles; exp on ACT -> P^T fp8e4
  - softmax denominator: quarters 0-1 via DVE bf16 chunk-tree,
    quarters 2-3 via fp8 DoubleRow ones-matmuls, all accumulated in
    one [1, q] psum group (the ones carry the 64x V scale)
  - reciprocals and rsqrt run on ACT as exp(-ln x) / exp(-0.5 ln x):
    Ln and Exp share one activation-table set, so no table switches
  - row -> all-partition replication via GpSimd partition broadcast
  - LN per-query-block, overlapped with the next block's attention
"""

import numpy as np

import concourse.bass as bass
import concourse.mybir as mybir
import concourse.tile as tile
from concourse import bacc
from concourse.bass import ds, ts
from concourse.bass_utils import run_bass_kernel_spmd

F32 = mybir.dt.float32
F32R = mybir.dt.float32r
BF16 = mybir.dt.bfloat16
F8 = mybir.dt.float8e4
AF = mybir.ActivationFunctionType
OP = mybir.AluOpType
PM = mybir.MatmulPerfMode

B = 4
C = 256
N = 4096          # tokens per batch
NQ = 2048         # queries per core
QB = 512          # query block
NQB = NQ // QB    # 4
NKC = N // 128    # 32 key chunks
NKR = 4           # key ranges (1024 keys each) for K^T / V tiles
SQK = 16.0        # host-side scale on wq/wk (fp8 subnormal avoidance)
SV = 64.0         # host-side scale on wvo
SCALE = (C // 8) ** -0.5 / (SQK * SQK)
LN_EPS = 1e-5

_CACHE = {}


def _build_nc(dbg=False):
    nc = bacc.Bacc("TRN2", target_bir_lowering=False, debug=False)

    low_d = nc.declare_dram_parameter("low", [C, NQ], F32R, isOutput=False)
    lowq_d = nc.declare_dram_parameter("lowq", [C, NQ], F8, isOutput=False)
    high_d = nc.declare_dram_parameter("high", [C, N], F8, isOutput=False)
    # weights pre-transposed [c_in, c_out]; wvo = (o_w @ v_w).T * 64
    wq_d = nc.declare_dram_parameter("wq", [C, C], F8, isOutput=False)
    wk_d = nc.declare_dram_parameter("wk", [C, C], F8, isOutput=False)
    wvo_d = nc.declare_dram_parameter("wvo", [C, C], F8, isOutput=False)
    # qb16, kb16, ob_eff, lng, lnb prepacked host-side as [128, 10]
    pvec_d = nc.declare_dram_parameter("pvec", [128, 10], F32, isOutput=False)
    out_d = nc.declare_dram_parameter("out", [C, NQ], F32, isOutput=True)
    dbg_d = {}
    if dbg:
        for nm, shp, dt_ in [
            ("dbg_rcp", [1, 512], F32), ("dbg_mu", [1, 512], F32),
            ("dbg_var", [1, 512], F32), ("dbg_rstd", [1, 512], F32),
            ("dbg_pps", [128, 2, QB], F32),
            ("dbg_qt", [128, 2, QB], F8), ("dbg_kt", [128, 2, 1024], F8),
            ("dbg_v", [128, 8, C], F8), ("dbg_pt", [128, 8, QB], F8),
        ]:
            dbg_d[nm] = nc.declare_dram_parameter(nm, shp, dt_, isOutput=True)

    with tile.TileContext(nc) as tc:
        with (
            tc.tile_pool(name="persist", bufs=1) as pp,
            tc.tile_pool(name="high", bufs=3) as high_pool,
            tc.tile_pool(name="pt", bufs=9) as pt_pool,
            tc.tile_pool(name="yt", bufs=2) as yt_pool,
            tc.tile_pool(name="scratch", bufs=2) as scr_pool,
            tc.tile_pool(name="rowscr", bufs=1) as row_pool,
            tc.tile_pool(name="outsb", bufs=2) as out_pool,
            tc.tile_pool(name="st_ps", bufs=2, space="PSUM") as st_ps,
            tc.tile_pool(name="acc_ps", bufs=3, space="PSUM") as acc_ps,
            tc.tile_pool(name="row_ps", bufs=1, space="PSUM") as row_ps,
        ):
            # ---------------- constants / parameters ----------------
            # single-descriptor DMAs: the [256, n] DRAM halves fold into
            # [128, 2, n] SBUF tiles via AP rearrange, one post each
            pvec = pp.tile([128, 10], F32)
            nc.sync.dma_start(out=pvec[:, :], in_=pvec_d[:, :])
            # DMA order matters: hi chunks first (gate the K proj and
            # everything after), then lowq; the big f32 low residual
            # rides the scalar queue after the weights (needed ~25us in)
            hi_sb = [
                pp.tile([128, 2, 1024], F8, name=f"hi{r}", tag=f"hi{r}")
                for r in range(NKR)
            ]
            wk_sb = pp.tile([128, 2, C], F8)
            wq_sb = pp.tile([128, 2, C], F8)
            wvo_sb = pp.tile([128, 2, C], F8)
            for t, d in [(wk_sb, wk_d), (wq_sb, wq_d), (wvo_sb, wvo_d)]:
                nc.scalar.dma_start(
                    out=t[:, :, :],
                    in_=d[:, :].rearrange("(j p) k -> p j k", j=2),
                )
            for r in range(NKR):
                nc.sync.dma_start(
                    out=hi_sb[r][:, :, :],
                    in_=high_d[:, ds(r * 1024, 1024)].rearrange(
                        "(j p) k -> p j k", j=2
                    ),
                )
            lowq_sb = pp.tile([128, 2, NQ], F8)
            nc.sync.dma_start(
                out=lowq_sb[:, :, :],
                in_=lowq_d[:, :].rearrange("(j p) k -> p j k", j=2),
            )
            low_sb = pp.tile([128, 2, NQ], F32R)
            nc.scalar.dma_start(
                out=low_sb[:, :, :],
                in_=low_d[:, :].rearrange("(j p) k -> p j k", j=2),
            )

            # memset cannot emit float32r/fp8; stage in f32 and copy
            stage = pp.tile([128, 128], F32)
            ones128 = pp.tile([128, 1], F32R)    # partition-reduce lhsT (f32r)
            nc.vector.memset(stage[:, 0:1], 1.0)
            nc.vector.tensor_copy(ones128[:, :], stage[:, 0:1])
            # denominator lhsT carries the 64x V scale
            ones_f8 = pp.tile([128, 2, 16], F8)  # fp8 DoubleRow ones (col 0)
            nc.vector.memset(stage[:, 0:32], SV)
            nc.vector.tensor_copy(
                ones_f8[:, :, :], stage[:, 0:32].rearrange("p (a b) -> p a b", a=2)
            )
            epsb = pp.tile([1, 1], F32)          # LN epsilon bias
            nc.vector.memset(epsb[:, :], LN_EPS)

            # HAM warm-up: dummy f32 matmuls fill the DMA-wait window so
            # the PE clock-gate opens before the first real matmul
            warm_ps = row_ps.tile([1, 128], F32, tag="row")
            for w in range(10):
                nc.tensor.matmul(
                    out=warm_ps[:, :], lhsT=stage[:, 0:1], rhs=stage[:, :],
                    start=True, stop=True,
                )

            QBIAS, KBIAS, OBIAS, LNG, LNB = 0, 2, 4, 6, 8

            # ---------------- projections ----------------
            # psum tiles alternate between the st/acc pools so the
            # projections pace at PE speed, not DVE bias-add speed.
            kt_sb = [
                pp.tile([128, 2, 1024], F8, name=f"kt{r}", tag=f"kt{r}")
                for r in range(NKR)
            ]
            v_sb = [
                pp.tile([128, 8, C], F8, name=f"v{r}", tag=f"v{r}")
                for r in range(NKR)
            ]
            qt_all = pp.tile([128, 2, NQ], F8)

            def proj_psum(i, shape):
                pool = (st_ps, acc_ps, acc_ps)[i % 3]
                return pool.tile(
                    shape, F32, tag="st" if i % 3 == 0 else "acc",
                    name=f"pps{i}",
                )

            def k_proj():
                # K^T: out [cout, k] = sum_cin wk[cin, cout] high[cin, k]
                i = 0
                for r in range(NKR):
                    for h in range(2):
                        for c in range(2):
                            kps = proj_psum(i, [128, 512])
                            i += 1
                            nc.tensor.matmul(
                                out=kps[:, :],
                                lhsT=wk_sb[:, :, ds(c * 128, 128)],
                                rhs=hi_sb[r][:, :, ds(h * 512, 512)],
                                start=True, stop=True,
                                perf_mode=PM.DoubleRow,
                            )
                            nc.vector.tensor_scalar_add(
                                out=kt_sb[r][:, c, ds(h * 512, 512)],
                                in0=kps[:, :],
                                scalar1=pvec[:, ds(KBIAS + c, 1)],
                            )

            def q_proj():
                i = 0
                for qb4 in range(NQB):
                    for c in range(2):
                        qps = proj_psum(i, [128, QB])
                        i += 1
                        nc.tensor.matmul(
                            out=qps[:, :],
                            lhsT=wq_sb[:, :, ds(c * 128, 128)],
                            rhs=lowq_sb[:, :, ds(qb4 * QB, QB)],
                            start=True, stop=True,
                            perf_mode=PM.DoubleRow,
                        )
                        nc.vector.tensor_scalar_add(
                            out=qt_all[:, c, ds(qb4 * QB, QB)], in0=qps[:, :],
                            scalar1=pvec[:, ds(QBIAS + c, 1)],
                        )

            def v_proj_step(i):
                # V': out [k, cout] = sum_cin high[cin, k] wvo[cin, cout]
                # one 128-key chunk, interleaved into attention(0) so it
                # fills PE idle under the exp stream; psums from the acc
                # pool (free until PV(0)); copies on DVE (ACT runs exp)
                r, u = i // 8, i % 8
                vps = acc_ps.tile([128, C], F32, tag="acc", name=f"vps{i}")
                nc.tensor.matmul(
                    out=vps[:, :],
                    lhsT=hi_sb[r][:, :, ds(u * 128, 128)],
                    rhs=wvo_sb[:, :, :],
                    start=True, stop=True,
                    perf_mode=PM.DoubleRow,
                )
                nc.vector.tensor_copy(v_sb[r][:, u, :], vps[:, :])

            # ---------------- main loop over query blocks ----------------

            def attention(b, filler=None):
                qsl = ds(b * QB, QB)
                quarters = [
                    pt_pool.tile([128, 8, QB], F8, tag="ptq", name=f"ptq{g}")
                    for g in range(4)
                ]
                for si in range(16):
                    sps = st_ps.tile([128, 2, QB], F32, tag="st")
                    for u in range(2):
                        kc = si * 2 + u
                        # DoubleRow: full C=256 contraction in one fp8 MM
                        nc.tensor.matmul(
                            out=sps[:, u, :],
                            lhsT=kt_sb[kc // 8][:, :, ds((kc % 8) * 128, 128)],
                            rhs=qt_all[:, :, qsl],
                            start=True, stop=True,
                            perf_mode=PM.DoubleRow,
                        )
                    nc.scalar.activation(
                        out=quarters[si // 4][:, ds((si % 4) * 2, 2), :],
                        in_=sps[:, :, :],
                        func=AF.Exp,
                        scale=SCALE,
                    )
                    if filler is not None:
                        filler(si)
                return quarters

            def pv(b, quarters):
                pps = []
                for c in range(2):
                    ops = acc_ps.tile([128, QB], F32, tag="acc")
                    for t in range(NKC // 2):
                        # DoubleRow: two adjacent 128-key chunks per fp8 MM
                        nc.tensor.matmul(
                            out=ops[:, :],
                            lhsT=v_sb[t // 4][:, ds((t % 4) * 2, 2), ds(c * 128, 128)],
                            rhs=quarters[t // 4][:, ds((t % 4) * 2, 2), :],
                            start=(t == 0), stop=(t == NKC // 2 - 1),
                            perf_mode=PM.DoubleRow,
                        )
                    pps.append(ops)
                return pps

            def denom(b, quarters):
                # softmax denominator: fp8 DoubleRow ones-matmuls over
                # every quarter pair, one [1, QB] psum accumulation
                # group; the 64x lhsT values fold in the V' scale.
                # Purely exp-gated (no DVE dependency), so the next
                # block's S matmuls aren't stalled behind DVE folds.
                dn_ps = row_ps.tile([1, QB], F32, tag="row")
                for i in range(16):
                    nc.tensor.matmul(
                        out=dn_ps[:, :],
                        lhsT=ones_f8[:, :, 0:1],
                        rhs=quarters[i // 4][:, ds((i % 4) * 2, 2), :],
                        start=(i == 0), stop=(i == 15),
                        perf_mode=PM.DoubleRow,
                    )
                # 1/denom = exp(-ln(denom)) on ACT (same table set as Exp)
                lnrow = row_pool.tile([1, QB], F32, tag="lnrow")
                nc.scalar.activation(
                    out=lnrow[:, :], in_=dn_ps[:, :], func=AF.Ln
                )
                rcprow = row_pool.tile([1, QB], F32, tag="rcprow",
                                       name=f"rcprow{b}")
                nc.scalar.activation(
                    out=rcprow[:, :], in_=lnrow[:, :], func=AF.Exp, scale=-1.0
                )
                rcp_rep = scr_pool.tile([128, QB], F32, tag="rcprep",
                                        name=f"rcprep{b}")
                nc.gpsimd.partition_broadcast(rcp_rep[:, :], rcprow[:, :])
                return rcprow, rcp_rep

            def make_y(b, pps, rcp_rep):
                qsl = ds(b * QB, QB)
                y_sb = yt_pool.tile([128, 2, QB], F32R, tag="y", name=f"y{b}")
                for c in range(2):
                    ysc = scr_pool.tile([128, QB], F32, tag="scr")
                    nc.vector.tensor_mul(
                        out=ysc[:, :], in0=pps[c][:, :], in1=rcp_rep[:, :]
                    )
                    nc.vector.scalar_tensor_tensor(
                        out=y_sb[:, c, :],
                        in0=ysc[:, :],
                        scalar=pvec[:, ds(OBIAS + c, 1)],
                        in1=low_sb[:, c, qsl].bitcast(F32),
                        op0=OP.add, op1=OP.add,
                    )
                return y_sb

            def stats_ln(b, y_sb, rcprow):
                sy_ps = row_ps.tile([1, QB], F32, tag="row")
                for c in range(2):
                    nc.tensor.matmul(
                        out=sy_ps[:, :],
                        lhsT=ones128[:, :],
                        rhs=y_sb[:, c, :],
                        start=(c == 0), stop=(c == 1),
                    )
                murow = row_pool.tile([1, QB], F32, tag="murow")
                nc.vector.tensor_scalar_mul(
                    out=murow[:, :], in0=sy_ps[:, :], scalar1=1.0 / C
                )
                sy2_ps = row_ps.tile([1, QB], F32, tag="row")
                for c in range(2):
                    ysq = scr_pool.tile([128, QB], F32R, tag="ysq")
                    nc.vector.tensor_mul(
                        out=ysq[:, :],
                        in0=y_sb[:, c, :].bitcast(F32),
                        in1=y_sb[:, c, :].bitcast(F32),
                    )
                    nc.tensor.matmul(
                        out=sy2_ps[:, :],
                        lhsT=ones128[:, :],
                        rhs=ysq[:, :],
                        start=(c == 0), stop=(c == 1),
                    )
                # var = E[y^2] - mu^2 ; rstd = exp(-0.5 ln(var + eps))
                varrow = row_pool.tile([1, QB], F32, tag="varrow")
                nc.vector.tensor_scalar_mul(
                    out=varrow[:, :], in0=sy2_ps[:, :], scalar1=1.0 / C
                )
                mu2row = row_pool.tile([1, QB], F32, tag="mu2row")
                nc.vector.tensor_mul(
                    out=mu2row[:, :], in0=murow[:, :], in1=murow[:, :],
                )
                nc.vector.tensor_sub(
                    out=varrow[:, :], in0=varrow[:, :], in1=mu2row[:, :]
                )
                lnv = row_pool.tile([1, QB], F32, tag="lnv")
                nc.scalar.activation(
                    out=lnv[:, :], in_=varrow[:, :], func=AF.Ln, bias=epsb[:, :]
                )
                rstdrow = row_pool.tile([1, QB], F32, tag="rstdrow")
                nc.scalar.activation(
                    out=rstdrow[:, :], in_=lnv[:, :], func=AF.Exp, scale=-0.5
                )
                if dbg_d and b == NQB - 1:
                    nc.sync.dma_start(out=dbg_d["dbg_rcp"][:, :], in_=rcprow[:, :])
                    nc.sync.dma_start(out=dbg_d["dbg_mu"][:, :], in_=murow[:, :])
                    nc.sync.dma_start(out=dbg_d["dbg_var"][:, :],
                                      in_=varrow[:, :])
                    nc.sync.dma_start(out=dbg_d["dbg_rstd"][:, :],
                                      in_=rstdrow[:, :])
                mu_rep = scr_pool.tile([128, QB], F32, tag="murep")
                nc.gpsimd.partition_broadcast(mu_rep[:, :], murow[:, :])
                rs_rep = scr_pool.tile([128, QB], F32, tag="rsrep")
                nc.gpsimd.partition_broadcast(rs_rep[:, :], rstdrow[:, :])
                qsl = ds(b * QB, QB)
                osb = out_pool.tile([128, 2, QB], F32)
                for c in range(2):
                    yn = scr_pool.tile([128, QB], F32, tag="scr")
                    nc.vector.tensor_sub(
                        out=yn[:, :],
                        in0=y_sb[:, c, :].bitcast(F32),
                        in1=mu_rep[:, :],
                    )
                    nc.vector.tensor_mul(
                        out=yn[:, :], in0=yn[:, :], in1=rs_rep[:, :]
                    )
                    nc.vector.tensor_scalar(
                        out=osb[:, c, :], in0=yn[:, :],
                        scalar1=pvec[:, ds(LNG + c, 1)],
                        scalar2=pvec[:, ds(LNB + c, 1)],
                        op0=OP.mult, op1=OP.add,
                    )
                nc.sync.dma_start(
                    out=out_d[:, qsl].rearrange("(j p) q -> p j q", j=2),
                    in_=osb[:, :, :],
                )

            k_proj()
            q_proj()

            def v_filler(si):
                v_proj_step(2 * si)
                v_proj_step(2 * si + 1)

            for b in range(NQB):
                quarters = attention(b, filler=v_filler if b == 0 else None)
                pps = pv(b, quarters)
                rcprow, rcp_rep = denom(b, quarters)
                y_b = make_y(b, pps, rcp_rep)
                stats_ln(b, y_b, rcprow)
                if dbg_d and b == NQB - 1:
                    nc.sync.dma_start(
                        out=dbg_d["dbg_pps"][:, 0, :], in_=pps[0][:, :]
                    )
                    nc.sync.dma_start(out=dbg_d["dbg_qt"][:, :, :],
                                      in_=qt_all[:, :, 3 * QB:4 * QB])
                    nc.sync.dma_start(
                        out=dbg_d["dbg_kt"][:, :, :], in_=kt_sb[0][:, :, :]
                    )
                    nc.sync.dma_start(
                        out=dbg_d["dbg_v"][:, :, :], in_=v_sb[0][:, :, :]
                    )
                    nc.sync.dma_start(
                        out=dbg_d["dbg_pt"][:, :, :], in_=quarters[3][:, :, :]
                    )

    # Force Exp and Ln to resolve to the one table set containing both
    # (the default chooser alternates exp_and_others <-> natural_log_exp,
    # paying a ~1.3us table load per switch, ~17 loads per kernel).
    import bass_rust as _br
    from concourse.hw_specs import get_activation_tables as _gat

    def _patched_act_loads():
        has_act = any(
            isinstance(i, mybir.InstActivation)
            for blk in nc.main_func.blocks for i in blk.instructions
        )
        if not has_act:
            return
        tables = []
        for name, fns in _gat(nc.m.arch).items():
            if name != "natural_log_exp_and_others":
                fns = fns - {AF.Exp, AF.Ln}
            tables.append((name, fns))
        _br.insert_act_table_loads(nc, tables)

    nc.insert_act_table_loads = _patched_act_loads
    nc.compile()
    return nc


def get_nc(dbg=False):
    key = "nc_dbg" if dbg else "nc"
    if key not in _CACHE:
        _CACHE[key] = _build_nc(dbg)
    return _CACHE[key]


def make_in_maps(low, high, q_w, q_b, k_w, k_b, v_w, v_b, o_w, o_b, ln_g, ln_b):
    import ml_dtypes
    f32 = lambda x: np.ascontiguousarray(np.asarray(x, np.float32))
    f8 = lambda x: np.ascontiguousarray(
        np.asarray(x, np.float32).astype(ml_dtypes.float8_e4m3)
    )
    low_r = np.asarray(low, np.float32).reshape(B, C, N)
    high_r = np.asarray(high, np.float32).reshape(B, C, N)
    # v-bias is exactly equivalent to an out-proj bias shift because the
    # softmax rows sum to one: attn @ (V + 1 vb^T) @ o_w^T = attn @ V @ o_w^T
    # + (o_w @ v_b)^T, so fold it on the host. The out-projection itself
    # folds into V: attn @ V @ o_w^T = attn @ (high_t @ (o_w @ v_w).T).
    o_w = np.asarray(o_w, np.float32)
    v_w = np.asarray(v_w, np.float32)
    ob_eff = np.asarray(o_b, np.float32) + o_w @ np.asarray(v_b, np.float32)
    w_vo = (o_w @ v_w) * SV
    pv_cols = []
    for v in [np.asarray(q_b, np.float32) * SQK,
              np.asarray(k_b, np.float32) * SQK, ob_eff, ln_g, ln_b]:
        pv_cols.append(np.asarray(v, np.float32).reshape(2, 128).T)
    shared = {
        "wq": f8(np.asarray(q_w, np.float32).T * SQK),
        "wk": f8(np.asarray(k_w, np.float32).T * SQK),
        "wvo": f8(w_vo.T),
        "pvec": f32(np.concatenate(pv_cols, axis=1)),
    }
    in_maps = []
    for i in range(8):
        bidx, h = i // 2, i % 2
        in_maps.append({
            "low": f32(low_r[bidx][:, h * NQ:(h + 1) * NQ]),
            "lowq": f8(low_r[bidx][:, h * NQ:(h + 1) * NQ]),
            "high": f8(high_r[bidx]),
            **shared,
        })
    return in_maps


def assemble(results):
    out = np.empty((B, C, N), np.float32)
    for i in range(8):
        bidx, h = i // 2, i % 2
        out[bidx][:, h * NQ:(h + 1) * NQ] = results[i]["out"]
    return out.reshape(B, C, 64, 64)


def kernel(**inputs) -> np.ndarray:
    nc = get_nc()
    in_maps = make_in_maps(**inputs)
    res = run_bass_kernel_spmd(nc, in_maps, core_ids=list(range(8)))
    return assemble(res.results)


if __name__ == "__main__":
    pass
